# revision 35
# baseline (speedup 1.0000x reference)
"""Trainium2 Bass kernel for single-head self-attention over image tokens.

Reference computation (per batch element b of 4):
    xf   = x[b] viewed as [N=4096 tokens, C=256]          (x stored [C, H*W] = xf.T)
    qkv  = xf @ w_qkv.T                                   -> q, k, v each [N, 512]
    sim  = (q * 64**-0.5) @ k.T                           [N, N]
    attn = softmax(sim, axis=-1)
    out  = (attn @ v) @ w_out.T + b_out + xf              [N, C]

Algebraic factorization (the key optimization): INNER=512 > C=256, so the
whole block collapses through two host-precomputed [256, 256] matrices
    M  = wq.T @ wk          (sim  = xf @ M @ xf.T, scaled at the exp)
    W2 = (w_out @ wv).T     (out  = softmax(...) @ xf @ W2 + b + xf)
eliminating the q/k/v projections entirely and halving both big
contractions (sim: 512 -> 256, attn@v: token-space @ xf instead of v).
Per-core PE work drops from ~432K to ~218K cycles.

Sharding: 8 cores = 4 batches x 2 query-row halves (2048 rows each). Each
core's x is host-rotated so its query half is always columns 0:2048. No
collectives.

Precision (numpy-simulated rel err 1.14e-2 vs the 2e-2 budget; the sim
predicted the previous kernel's hw error to 3 digits):
  - tT = M.T @ x: bf16 inputs, f32 PSUM, tT stored bf16.
  - sim contraction (256 dims) in bf16: stationary = resident x chunks,
    moving = tT. Full-fp8 sim measures 2.27e-2 -- over budget.
  - pT = exp(0.125*sim - 7) in fp8e5 (range); attn@xf as fp8 DoubleRow
    over token-pair planes with x in fp8e4 (host-prepared pair layout),
    f32 PSUM accumulated across ALL 4096 keys in 2 persistent PSUM banks
    per c-chunk (no SBUF accumulator, no per-superblock drain).
  - out projection W2 in bf16; residual read from the resident bf16 x.

Schedule: one pass per 512-query slice over all 32 key chunks. exp rides
the ACT engine (~14us/slice) under the PE's ~22us/slice; the softmax
denominator tree-sums alternate between DVE and Pool; each slice's
finalize (1/l via ones-matmul colsum, W2 projection, bias+residual, out
DMA) is deferred into the next slice's PE stream. The last slice handles
its final two token-pairs' denominator on the PE (DoubleRow ones-matmul)
so the kernel tail never waits on the adder tree, and finalizes in two
256-col halves with DMAs spread across queues.
"""

import hashlib
import os
import shutil

import numpy as np

import concourse.bacc as bacc
import concourse.tile as tile
import concourse.mybir as mybir
from concourse.bass_utils import run_bass_kernel_spmd


def _install_neff_cache():
    """Disk-cache walrus NEFF compiles keyed on the BIR content hash.

    The axon PJRT path recompiles the NEFF in every fresh process (~minutes);
    the build here is deterministic, so identical BIR -> identical NEFF.
    """
    try:
        import concourse.bass2jax as bass2jax
        orig = bass2jax.compile_bir_kernel
        if getattr(orig, "_neff_cache_wrapped", False):
            return
        cache_dir = os.path.expanduser("~/.neuron-compile-cache/bass-neff")

        def cached(bir_json, tmpdir, neff_name="file.neff"):
            try:
                key = hashlib.sha256(
                    bir_json if isinstance(bir_json, bytes)
                    else bir_json.encode()).hexdigest()
                hit = os.path.join(cache_dir, key + ".neff")
                dst = os.path.join(tmpdir, neff_name)
                if os.path.exists(hit):
                    shutil.copyfile(hit, dst)
                    return dst
                neff = orig(bir_json, tmpdir, neff_name=neff_name)
                os.makedirs(cache_dir, exist_ok=True)
                tmp = hit + ".tmp%d" % os.getpid()
                shutil.copyfile(neff, tmp)
                os.replace(tmp, hit)
                return neff
            except Exception:
                return orig(bir_json, tmpdir, neff_name=neff_name)

        cached._neff_cache_wrapped = True
        bass2jax.compile_bir_kernel = cached
    except Exception:
        pass


_install_neff_cache()

F32 = mybir.dt.float32
F32R = mybir.dt.float32r
BF16 = mybir.dt.bfloat16
F8E4 = mybir.dt.float8e4
F8E5 = mybir.dt.float8e5
DR = mybir.MatmulPerfMode.DoubleRow
Exp = mybir.ActivationFunctionType.Exp
Add = mybir.AluOpType.add
Mult = mybir.AluOpType.mult
U8 = mybir.dt.uint8
# Schraudolph fast-exp: e5m2 bits = round(4*log2(e)*(SCALE*sim - SHIFT) + 60
# - 4*C); C=0.0573 minimizes end-to-end rel err (numpy-swept)
EXPA = 4 * 0.125 * 1.4426950408889634
EXPB = 60 - 4 * 7 * 1.4426950408889634 - 4 * 0.0573
SHIFT = 7.0  # exp(scale*sim - SHIFT): keeps pT < e5m2 max; cancels in out

B = 4
C = 256          # model dim (2 chunks of 128)
N = 4096         # tokens per batch (64*64)
HALF = N // 2    # query rows per core
SCALE = 0.125    # 64 ** -0.5

NCORES = 8
NSL = 4          # query slices per core
SW = HALF // NSL # 512 query columns per slice
NJ8 = N // 128   # 32 key chunks
NPAIR = NJ8 // 2 # 16 key token-pairs (256 keys each)


def build_nc():
    nc = bacc.Bacc(None)
    x_r = nc.declare_dram_parameter("x_r", [C, N], BF16, isOutput=False)
    xq8_d = nc.declare_dram_parameter("xq8", [128, 2, N], F8E4,
                                      isOutput=False)
    xp_d = nc.declare_dram_parameter("xp", [128, NPAIR, 2, C], F8E4,
                                     isOutput=False)
    m_d = nc.declare_dram_parameter("m", [C, C], BF16, isOutput=False)
    w2_d = nc.declare_dram_parameter("w2", [C, C], BF16, isOutput=False)
    bout = nc.declare_dram_parameter("bout", [2, 128, 1], F32, isOutput=False)
    out = nc.declare_dram_parameter("out", [C, HALF], BF16, isOutput=True)

    mm = nc.tensor.matmul

    with tile.TileContext(nc) as tc:
        with tc.tile_pool(name="const", bufs=1) as const, \
             tc.tile_pool(name="work", bufs=2) as work, \
             tc.tile_pool(name="pp", bufs=1, space="PSUM") as pp:

            # ---- resident inputs ----
            ones_sq_f = const.tile([128, 128], F32, tag="ones_sq_f",
                                   name="ones_sq_f")
            nc.vector.memset(ones_sq_f, 1.0)

            # M first (tiny, sync queue) so tT mms only wait on x pieces
            mt = []
            for cc in range(2):
                t = const.tile([128, C], BF16, tag=f"m{cc}", name=f"m{cc}")
                nc.sync.dma_start(t, m_d[cc * 128:(cc + 1) * 128, :])
                mt.append(t)
            # x [C, N] bf16: small leading piece so tT starts early,
            # larger trailing pieces; 2 queues
            xr = [const.tile([128, N], BF16, tag=f"xr{cc}", name=f"xr{cc}")
                  for cc in range(2)]
            for (col, w) in ((0, 512), (512, 1024), (1536, 1024),
                             (2560, 1536)):
                for cc in range(2):
                    eng = nc.gpsimd if cc == 0 else nc.scalar
                    eng.dma_start(
                        xr[cc][:, col:col + w],
                        x_r[cc * 128:(cc + 1) * 128, col:col + w])
            # x token-pair planes fp8e4 [128, pair, plane, C], 4 pieces
            xpt = const.tile([128, NPAIR, 2, C], F8E4, tag="xpt", name="xpt")
            for piece in range(4):
                p0 = piece * (NPAIR // 4)
                nc.sync.dma_start(xpt[:, p0:p0 + NPAIR // 4, :, :],
                                  xp_d[:, p0:p0 + NPAIR // 4, :, :])
            # fp8 c-pair-plane copy of x for the DoubleRow sim key pairs
            xq8 = const.tile([128, 2, N], F8E4, tag="xq8", name="xq8")
            for (col, w) in ((0, 1024), (1024, 1024), (2048, 1024),
                             (3072, 1024)):
                eng = nc.gpsimd if col % 2048 == 0 else nc.scalar
                eng.dma_start(xq8[:, :, col:col + w],
                              xq8_d[:, :, col:col + w])

            tt = [const.tile([128, HALF], BF16, tag=f"tt{cc}", name=f"tt{cc}")
                  for cc in range(2)]
            tp8 = const.tile([128, 2, HALF], F8E4, tag="tp8", name="tp8")

            # ---- PE warm-up: ~5us of dependency-free bf16 matmuls so the
            # HAM clock gate reaches K=8/8 (2.4 GHz) before real work,
            # regardless of input-DMA latency (the gate needs ~3.4us of
            # SUSTAINED busy; f32r warm-ups measured as not tripping it).
            ones_bf = const.tile([128, 512], BF16, tag="ones_bf",
                                 name="ones_bf")
            ones_bf_f = const.tile([128, 512], F32, tag="ones_bf_f",
                                   name="ones_bf_f")
            nc.vector.memset(ones_bf_f, 1.0)
            nc.vector.tensor_copy(ones_bf, ones_bf_f)
            warm_ps = pp.tile([128, 512], F32, tag="aux", bufs=2,
                              name="warm_ps")
            NWARM = 14
            for i in range(NWARM):
                mm(warm_ps, ones_bf[:, 0:128], ones_bf,
                   start=(i == 0), stop=(i == NWARM - 1))
            # pre-trigger the ACT exp table load (~2.7us) during the
            # input DMA wait so the first real exp doesn't pay it
            warm_act = const.tile([128, 1], F32, tag="warm_act",
                                  name="warm_act")
            nc.scalar.activation(warm_act, ones_sq_f[:, 0:1], Exp,
                                 scale=1.0)
            # all-ones fp8 pair plane for the tail DoubleRow colsum
            ones_psq = const.tile([128, 2, 128], F8E4, tag="ones_psq",
                                  name="ones_psq")
            nc.gpsimd.tensor_copy(ones_psq[:, 0, :], ones_sq_f)
            nc.gpsimd.tensor_copy(ones_psq[:, 1, :], ones_sq_f)
            nshift = const.tile([128, 1], F32, tag="nshift", name="nshift")
            nc.vector.memset(nshift, -SHIFT)

            # ---- tT = M.T @ x for query columns 0:HALF ----
            ncopy = 0
            for s in range(NSL):
                sl = slice(s * SW, (s + 1) * SW)
                for co in range(2):
                    ps = pp.tile([128, SW], F32, tag="sim", bufs=3,
                                 name="ps_t")
                    for cc in range(2):
                        mm(ps, mt[cc][:, co * 128:(co + 1) * 128],
                           xr[cc][:, sl], start=(cc == 0), stop=(cc == 1))
                    # alternate the psum->bf16/fp8 copies over ACT/DVE
                    # (Pool can't access PSUM)
                    if ncopy % 2 == 0:
                        nc.scalar.copy(tt[co][:, sl], ps)
                        nc.vector.tensor_copy(tp8[:, co, sl], ps)
                    else:
                        nc.vector.tensor_copy(tt[co][:, sl], ps)
                        nc.scalar.copy(tp8[:, co, sl], ps)
                    ncopy += 1
            # final-phase constants, off the startup critical path
            w2t = []
            for cc in range(2):
                t = const.tile([128, C], BF16, tag=f"w2{cc}", name=f"w2{cc}")
                nc.sync.dma_start(t, w2_d[cc * 128:(cc + 1) * 128, :])
                w2t.append(t)
            bt = []
            for cc in range(2):
                t = const.tile([128, 1], F32, tag=f"b{cc}", name=f"b{cc}")
                nc.sync.dma_start(t, bout[cc])
                bt.append(t)

            # ---- attention: one pass per query slice over all keys ----
            deferred = []   # prev slice finalize part a (into this PE stream)
            deferred2 = []  # prev slice finalize part b
            for s in range(NSL):
                sl = slice(s * SW, (s + 1) * SW)
                tail = s == NSL - 1
                po = [pp.tile([128, SW], F32, tag=f"po{cc}", bufs=1,
                              name=f"po{cc}") for cc in range(2)]
                # softmax denominator: accumulated entirely on the PE via
                # DoubleRow all-ones colsum matmuls (broadcast to all 128
                # partitions), one per pair -- the DVE/ACT engines have a
                # ~2.3x silicon slowdown, so no adder tree off-PE
                pb = pp.tile([128, SW], F32, tag="aux", bufs=2, name="pb")
                ptp = []

                def drain_pair(p, po=po, pb=pb):
                    # attn@xf for token-pair p: fp8 DoubleRow, 256 keys
                    # contracted per mm, accumulating over all 16 pairs
                    for cc in range(2):
                        mm(po[cc], xpt[:, p, :, cc * 128:(cc + 1) * 128],
                           ptp[p][:, :, :],
                           start=(p == 0), stop=(p == NPAIR - 1),
                           perf_mode=DR)
                    mm(pb, ones_psq, ptp[p][:, :, :],
                       start=(p == 0), stop=(p == NPAIR - 1),
                       perf_mode=DR)

                for j8 in range(NJ8):
                    ps = pp.tile([128, SW], F32, tag="sim", bufs=3,
                                 name="ps_s")
                    fp8_pair = (j8 // 2) % 2 == 0
                    if fp8_pair:
                        # even key pairs: fp8 DoubleRow, 256-dim contraction
                        # in one mm (rel-err budget allows half the keys)
                        mm(ps, xq8[:, :, j8 * 128:(j8 + 1) * 128],
                           tp8[:, :, sl], start=True, stop=True,
                           perf_mode=DR)
                    else:
                        for cc in range(2):
                            mm(ps, xr[cc][:, j8 * 128:(j8 + 1) * 128],
                               tt[cc][:, sl], start=(cc == 0), stop=(cc == 1))
                    p, parity = divmod(j8, 2)
                    if parity == 0:
                        t = work.tile([128, 2, SW], F8E5, tag="pt", bufs=4,
                                      name="pt")
                        ptp.append(t)
                    if fp8_pair:
                        # Schraudolph fast-exp on the DVE: affine map into
                        # the e5m2 bit pattern (uint8 convert saturates);
                        # splits the exp load across ACT and DVE
                        nc.vector.tensor_scalar(
                            ptp[p][:, parity, :].bitcast(U8), ps,
                            EXPA, EXPB, op0=Mult, op1=Add)
                    else:
                        nc.scalar.activation(ptp[p][:, parity, :], ps, Exp,
                                             scale=SCALE, bias=nshift)
                    if parity == 1 and p > 0:
                        drain_pair(p - 1)
                    if j8 == 2:
                        for fn in deferred:
                            fn()
                        deferred.clear()
                    if j8 == 8:
                        for fn in deferred2:
                            fn()
                        deferred2.clear()
                drain_pair(NPAIR - 1)

                # ---- finalize slice s: normalize + project + out ----
                def emit_otr(off, wdt, s=s, po=po):
                    otr = [work.tile([128, wdt], BF16, tag=f"otr{cc}",
                                     bufs=2, name=f"otr{cc}")
                           for cc in range(2)]
                    # po is PSUM: only DVE/ACT may read it; keep ACT free
                    nc.vector.tensor_copy(otr[0], po[0][:, off:off + wdt])
                    nc.vector.tensor_copy(otr[1], po[1][:, off:off + wdt])
                    return otr

                def make_finalize(off, wdt, otr, s=s, pb=pb, dma_engs=None):
                    state = {}

                    def fin_cc(cc):
                        sl2 = slice(s * SW + off, s * SW + off + wdt)
                        pf = pp.tile([128, wdt], F32, tag="sim", bufs=3,
                                     name="pf")
                        for ci in range(2):
                            mm(pf, w2t[ci][:, cc * 128:(cc + 1) * 128],
                               otr[ci], start=(ci == 0), stop=(ci == 1))
                        fo = work.tile([128, wdt], F32, tag="fo", bufs=2,
                                       name="fo")
                        nc.vector.tensor_mul(fo, pf, state["bc"])
                        # bias+residual on the (otherwise idle) Pool engine
                        fo1 = work.tile([128, wdt], F32, tag="fo1", bufs=2,
                                        name="fo1")
                        nc.gpsimd.tensor_add(fo1, xr[cc][:, sl2], fo)
                        fo2 = work.tile([128, wdt], BF16, tag="fo2", bufs=2,
                                        name="fo2")
                        nc.gpsimd.tensor_scalar_add(fo2, fo1, bt[cc])
                        deng = dma_engs[cc] if dma_engs else nc.sync
                        deng.dma_start(out[cc * 128:(cc + 1) * 128, sl2], fo2)

                    def fin_a():
                        bc = work.tile([128, wdt], F32, tag="bc", bufs=2,
                                       name="bc")
                        rsc = work.tile([128, wdt], F32, tag="rsc", bufs=2,
                                        name="rsc")
                        nc.vector.reciprocal_approx_accurate(
                            bc, pb[:, off:off + wdt], rsc)
                        state["bc"] = bc
                        fin_cc(0)

                    def fin_b():
                        fin_cc(1)
                    return fin_a, fin_b

                if not tail:
                    otr = emit_otr(0, SW)
                    fa, fb = make_finalize(0, SW, otr)
                    deferred.append(fa)
                    deferred2.append(fb)
                else:
                    # kernel tail: two 256-col halves so the first half's
                    # projection/output overlaps the second's
                    hwd = SW // 2
                    otr0 = emit_otr(0, hwd)
                    otr1 = emit_otr(hwd, hwd)
                    a0, b0 = make_finalize(0, hwd, otr0,
                                           dma_engs=(nc.scalar, nc.sync))
                    a1, b1 = make_finalize(hwd, hwd, otr1,
                                           dma_engs=(nc.gpsimd, nc.scalar))
                    a0(); b0(); a1(); b1()

    nc.finalize()
    return nc


_NC_CACHE = None


def _get_nc():
    global _NC_CACHE
    if _NC_CACHE is None:
        _NC_CACHE = build_nc()
    return _NC_CACHE


def prepare_in_maps(x, w_qkv, w_out, b_out):
    x = np.asarray(x, dtype=np.float32)
    w_qkv = np.asarray(w_qkv, dtype=np.float32)
    w_out = np.asarray(w_out, dtype=np.float32)
    b_out = np.asarray(b_out, dtype=np.float32)

    import ml_dtypes
    bf16 = ml_dtypes.bfloat16
    f8e4 = ml_dtypes.float8_e4m3
    wq, wk, wv = w_qkv[:512], w_qkv[512:1024], w_qkv[1024:]
    M = (wq.T.astype(np.float64) @ wk.astype(np.float64)).astype(np.float32)
    W2 = (w_out.astype(np.float64) @ wv.astype(np.float64)).T.astype(np.float32)
    m_bf = np.ascontiguousarray(M).astype(bf16)
    w2_bf = np.ascontiguousarray(W2).astype(bf16)
    bout = np.ascontiguousarray(b_out.reshape(2, 128, 1))

    xr = x.reshape(B, C, N)
    in_maps = []
    for c in range(NCORES):
        b, h = divmod(c, 2)
        if h == 0:
            x_rot = xr[b]
        else:  # rotate so this core's query half sits in columns 0:HALF
            x_rot = np.concatenate([xr[b][:, HALF:], xr[b][:, :HALF]], axis=1)
        x_bf = x_rot.astype(bf16)
        # token-pair planes: xp[p, pair, plane, c] = xf[(pair*2+plane)*128+p, c]
        xf8 = np.ascontiguousarray(x_bf.T).astype(f8e4)          # [N, C]
        xp = np.ascontiguousarray(
            xf8.reshape(NPAIR, 2, 128, C).transpose(2, 0, 1, 3))  # [128,16,2,C]
        # c-pair planes: xq8[p, r, j] = fp8(x_rot[r*128+p, j])
        xq8 = np.ascontiguousarray(
            x_bf.astype(f8e4).reshape(2, 128, N).transpose(1, 0, 2))
        in_maps.append({
            "x_r": x_bf,
            "xq8": xq8,
            "xp": xp,
            "m": m_bf,
            "w2": w2_bf,
            "bout": bout,
        })
    return in_maps


def postprocess(results):
    outs = [results[c]["out"] for c in range(NCORES)]
    full = np.stack([np.concatenate([outs[2 * b], outs[2 * b + 1]], axis=1)
                     for b in range(B)])               # [B, C, N]
    return full.reshape(B, C, 64, 64).astype(np.float32)


def kernel(x, w_qkv, w_out, b_out):
    in_maps = prepare_in_maps(x, w_qkv, w_out, b_out)
    res = run_bass_kernel_spmd(_get_nc(), in_maps, core_ids=list(range(NCORES)))
    return postprocess(res.results)


# revision 36
# speedup vs baseline: 1.1301x; 1.1301x over previous
"""Trainium2 Bass kernel for single-head self-attention over image tokens.

Reference computation (per batch element b of 4):
    xf   = x[b] viewed as [N=4096 tokens, C=256]          (x stored [C, H*W] = xf.T)
    qkv  = xf @ w_qkv.T                                   -> q, k, v each [N, 512]
    sim  = (q * 64**-0.5) @ k.T                           [N, N]
    attn = softmax(sim, axis=-1)
    out  = (attn @ v) @ w_out.T + b_out + xf              [N, C]

Algebraic factorization (the key optimization): INNER=512 > C=256, so the
whole block collapses through two host-precomputed [256, 256] matrices
    M  = wq.T @ wk          (sim  = xf @ M @ xf.T, scaled at the exp)
    W2 = (w_out @ wv).T     (out  = softmax(...) @ xf @ W2 + b + xf)
eliminating the q/k/v projections entirely and halving both big
contractions (sim: 512 -> 256, attn@v: token-space @ xf instead of v).
Per-core PE work drops from ~432K to ~218K cycles.

Sharding: 8 cores = 4 batches x 2 query-row halves (2048 rows each). Each
core's x is host-rotated so its query half is always columns 0:2048. No
collectives.

Precision (numpy-simulated rel err 1.14e-2 vs the 2e-2 budget; the sim
predicted the previous kernel's hw error to 3 digits):
  - tT = M.T @ x: bf16 inputs, f32 PSUM, tT stored bf16.
  - sim contraction (256 dims) in bf16: stationary = resident x chunks,
    moving = tT. Full-fp8 sim measures 2.27e-2 -- over budget.
  - pT = exp(0.125*sim - 7) in fp8e5 (range); attn@xf as fp8 DoubleRow
    over token-pair planes with x in fp8e4 (host-prepared pair layout),
    f32 PSUM accumulated across ALL 4096 keys in 2 persistent PSUM banks
    per c-chunk (no SBUF accumulator, no per-superblock drain).
  - out projection W2 in bf16; residual read from the resident bf16 x.

Schedule: one pass per 512-query slice over all 32 key chunks. exp rides
the ACT engine (~14us/slice) under the PE's ~22us/slice; the softmax
denominator tree-sums alternate between DVE and Pool; each slice's
finalize (1/l via ones-matmul colsum, W2 projection, bias+residual, out
DMA) is deferred into the next slice's PE stream. The last slice handles
its final two token-pairs' denominator on the PE (DoubleRow ones-matmul)
so the kernel tail never waits on the adder tree, and finalizes in two
256-col halves with DMAs spread across queues.
"""

import hashlib
import os
import shutil

import numpy as np

import concourse.bacc as bacc
import concourse.tile as tile
import concourse.mybir as mybir
from concourse.bass_utils import run_bass_kernel_spmd


def _install_neff_cache():
    """Disk-cache walrus NEFF compiles keyed on the BIR content hash.

    The axon PJRT path recompiles the NEFF in every fresh process (~minutes);
    the build here is deterministic, so identical BIR -> identical NEFF.
    """
    try:
        import concourse.bass2jax as bass2jax
        orig = bass2jax.compile_bir_kernel
        if getattr(orig, "_neff_cache_wrapped", False):
            return
        cache_dir = os.path.expanduser("~/.neuron-compile-cache/bass-neff")

        def cached(bir_json, tmpdir, neff_name="file.neff"):
            try:
                key = hashlib.sha256(
                    bir_json if isinstance(bir_json, bytes)
                    else bir_json.encode()).hexdigest()
                hit = os.path.join(cache_dir, key + ".neff")
                dst = os.path.join(tmpdir, neff_name)
                if os.path.exists(hit):
                    shutil.copyfile(hit, dst)
                    return dst
                neff = orig(bir_json, tmpdir, neff_name=neff_name)
                os.makedirs(cache_dir, exist_ok=True)
                tmp = hit + ".tmp%d" % os.getpid()
                shutil.copyfile(neff, tmp)
                os.replace(tmp, hit)
                return neff
            except Exception:
                return orig(bir_json, tmpdir, neff_name=neff_name)

        cached._neff_cache_wrapped = True
        bass2jax.compile_bir_kernel = cached
    except Exception:
        pass


_install_neff_cache()

F32 = mybir.dt.float32
F32R = mybir.dt.float32r
BF16 = mybir.dt.bfloat16
F8E4 = mybir.dt.float8e4
F8E5 = mybir.dt.float8e5
DR = mybir.MatmulPerfMode.DoubleRow
Exp = mybir.ActivationFunctionType.Exp
Add = mybir.AluOpType.add
Mult = mybir.AluOpType.mult
U8 = mybir.dt.uint8
# Schraudolph fast-exp: e5m2 bits = round(4*log2(e)*(SCALE*sim - SHIFT) + 60
# - 4*C); C=0.0573 minimizes end-to-end rel err (numpy-swept)
EXPA = 4 * 0.125 * 1.4426950408889634
EXPB = 60 - 4 * 7 * 1.4426950408889634 - 4 * 0.0573
SHIFT = 7.0  # exp(scale*sim - SHIFT): keeps pT < e5m2 max; cancels in out

B = 4
C = 256          # model dim (2 chunks of 128)
N = 4096         # tokens per batch (64*64)
HALF = N // 2    # query rows per core
SCALE = 0.125    # 64 ** -0.5

NCORES = 8
NSL = 4          # query slices per core
SW = HALF // NSL # 512 query columns per slice
NJ8 = N // 128   # 32 key chunks
NPAIR = NJ8 // 2 # 16 key token-pairs (256 keys each)


def build_nc():
    nc = bacc.Bacc(None)
    x_r = nc.declare_dram_parameter("x_r", [C, N], BF16, isOutput=False)
    xq8_d = nc.declare_dram_parameter("xq8", [128, 2, N], F8E4,
                                      isOutput=False)
    xp_d = nc.declare_dram_parameter("xp", [128, NPAIR, 2, C], F8E4,
                                     isOutput=False)
    m_d = nc.declare_dram_parameter("m", [C, C], BF16, isOutput=False)
    w2_d = nc.declare_dram_parameter("w2", [C, C], BF16, isOutput=False)
    bout = nc.declare_dram_parameter("bout", [2, 128, 1], F32, isOutput=False)
    out = nc.declare_dram_parameter("out", [C, HALF], BF16, isOutput=True)

    mm = nc.tensor.matmul

    with tile.TileContext(nc) as tc:
        with tc.tile_pool(name="const", bufs=1) as const, \
             tc.tile_pool(name="work", bufs=2) as work, \
             tc.tile_pool(name="pp", bufs=1, space="PSUM") as pp:

            # ---- resident inputs ----
            ones_sq_f = const.tile([128, 128], F32, tag="ones_sq_f",
                                   name="ones_sq_f")
            nc.vector.memset(ones_sq_f, 1.0)

            # M first (tiny, sync queue) so tT mms only wait on x pieces
            mt = []
            for cc in range(2):
                t = const.tile([128, C], BF16, tag=f"m{cc}", name=f"m{cc}")
                nc.sync.dma_start(t, m_d[cc * 128:(cc + 1) * 128, :])
                mt.append(t)
            # x [C, N] bf16: small leading piece so tT starts early,
            # larger trailing pieces; 2 queues
            xr = [const.tile([128, N], BF16, tag=f"xr{cc}", name=f"xr{cc}")
                  for cc in range(2)]
            for (col, w) in ((0, 512), (512, 1024), (1536, 1024),
                             (2560, 1536)):
                for cc in range(2):
                    eng = nc.gpsimd if cc == 0 else nc.scalar
                    eng.dma_start(
                        xr[cc][:, col:col + w],
                        x_r[cc * 128:(cc + 1) * 128, col:col + w])
            # x token-pair planes fp8e4 [128, pair, plane, C], 4 pieces
            xpt = const.tile([128, NPAIR, 2, C], F8E4, tag="xpt", name="xpt")
            for piece in range(4):
                p0 = piece * (NPAIR // 4)
                nc.sync.dma_start(xpt[:, p0:p0 + NPAIR // 4, :, :],
                                  xp_d[:, p0:p0 + NPAIR // 4, :, :])
            # fp8 c-pair-plane copy of x for the DoubleRow sim key pairs
            xq8 = const.tile([128, 2, N], F8E4, tag="xq8", name="xq8")
            for (col, w) in ((0, 1024), (1024, 1024), (2048, 1024),
                             (3072, 1024)):
                eng = nc.gpsimd if col % 2048 == 0 else nc.scalar
                eng.dma_start(xq8[:, :, col:col + w],
                              xq8_d[:, :, col:col + w])

            tt = [const.tile([128, HALF], BF16, tag=f"tt{cc}", name=f"tt{cc}")
                  for cc in range(2)]
            tp8 = const.tile([128, 2, HALF], F8E4, tag="tp8", name="tp8")

            # ---- PE warm-up: ~5us of dependency-free bf16 matmuls so the
            # HAM clock gate reaches K=8/8 (2.4 GHz) before real work,
            # regardless of input-DMA latency (the gate needs ~3.4us of
            # SUSTAINED busy; f32r warm-ups measured as not tripping it).
            ones_bf = const.tile([128, 512], BF16, tag="ones_bf",
                                 name="ones_bf")
            ones_bf_f = const.tile([128, 512], F32, tag="ones_bf_f",
                                   name="ones_bf_f")
            nc.vector.memset(ones_bf_f, 1.0)
            nc.vector.tensor_copy(ones_bf, ones_bf_f)
            warm_ps = pp.tile([128, 512], F32, tag="aux", bufs=2,
                              name="warm_ps")
            NWARM = 14
            for i in range(NWARM):
                mm(warm_ps, ones_bf[:, 0:128], ones_bf,
                   start=(i == 0), stop=(i == NWARM - 1))
            # pre-trigger the ACT exp table load (~2.7us) during the
            # input DMA wait so the first real exp doesn't pay it
            warm_act = const.tile([128, 1], F32, tag="warm_act",
                                  name="warm_act")
            nc.scalar.activation(warm_act, ones_sq_f[:, 0:1], Exp,
                                 scale=1.0)
            # all-ones fp8 pair plane for the tail DoubleRow colsum
            ones_psq = const.tile([128, 2, 128], F8E4, tag="ones_psq",
                                  name="ones_psq")
            nc.gpsimd.tensor_copy(ones_psq[:, 0, :], ones_sq_f)
            nc.gpsimd.tensor_copy(ones_psq[:, 1, :], ones_sq_f)
            nshift = const.tile([128, 1], F32, tag="nshift", name="nshift")
            nc.vector.memset(nshift, -SHIFT)

            # ---- tT = M.T @ x for query columns 0:HALF ----
            ncopy = 0
            for s in range(NSL):
                sl = slice(s * SW, (s + 1) * SW)
                for co in range(2):
                    ps = pp.tile([128, SW], F32, tag="sim", bufs=3,
                                 name="ps_t")
                    for cc in range(2):
                        mm(ps, mt[cc][:, co * 128:(co + 1) * 128],
                           xr[cc][:, sl], start=(cc == 0), stop=(cc == 1))
                    # alternate the psum->bf16/fp8 copies over ACT/DVE
                    # (Pool can't access PSUM)
                    if ncopy % 2 == 0:
                        nc.scalar.copy(tt[co][:, sl], ps)
                        nc.vector.tensor_copy(tp8[:, co, sl], ps)
                    else:
                        nc.vector.tensor_copy(tt[co][:, sl], ps)
                        nc.scalar.copy(tp8[:, co, sl], ps)
                    ncopy += 1
            # final-phase constants, off the startup critical path
            w2t = []
            for cc in range(2):
                t = const.tile([128, C], BF16, tag=f"w2{cc}", name=f"w2{cc}")
                nc.sync.dma_start(t, w2_d[cc * 128:(cc + 1) * 128, :])
                w2t.append(t)
            bt = []
            for cc in range(2):
                t = const.tile([128, 1], F32, tag=f"b{cc}", name=f"b{cc}")
                nc.sync.dma_start(t, bout[cc])
                bt.append(t)

            # ---- attention: one pass per query slice over all keys ----
            deferred = []   # prev slice finalize part a (into this PE stream)
            deferred2 = []  # prev slice finalize part b
            for s in range(NSL):
                sl = slice(s * SW, (s + 1) * SW)
                tail = s == NSL - 1
                po = [pp.tile([128, SW], F32, tag=f"po{cc}", bufs=1,
                              name=f"po{cc}") for cc in range(2)]
                # softmax denominator: accumulated entirely on the PE via
                # DoubleRow all-ones colsum matmuls (broadcast to all 128
                # partitions), one per pair -- the DVE/ACT engines have a
                # ~2.3x silicon slowdown, so no adder tree off-PE
                pb = pp.tile([128, SW], F32, tag="aux", bufs=2, name="pb")
                ptp = []

                def drain_pair(p, po=po, pb=pb):
                    # attn@xf for token-pair p: fp8 DoubleRow, 256 keys
                    # contracted per mm, accumulating over all 16 pairs
                    for cc in range(2):
                        mm(po[cc], xpt[:, p, :, cc * 128:(cc + 1) * 128],
                           ptp[p][:, :, :],
                           start=(p == 0), stop=(p == NPAIR - 1),
                           perf_mode=DR)
                    mm(pb, ones_psq, ptp[p][:, :, :],
                       start=(p == 0), stop=(p == NPAIR - 1),
                       perf_mode=DR)

                for j8 in range(NJ8):
                    ps = pp.tile([128, SW], F32, tag="sim", bufs=3,
                                 name="ps_s")
                    fp8_pair = (j8 // 2) % 2 == 0
                    if fp8_pair:
                        # even key pairs: fp8 DoubleRow, 256-dim contraction
                        # in one mm (rel-err budget allows half the keys)
                        mm(ps, xq8[:, :, j8 * 128:(j8 + 1) * 128],
                           tp8[:, :, sl], start=True, stop=True,
                           perf_mode=DR)
                    else:
                        for cc in range(2):
                            mm(ps, xr[cc][:, j8 * 128:(j8 + 1) * 128],
                               tt[cc][:, sl], start=(cc == 0), stop=(cc == 1))
                    p, parity = divmod(j8, 2)
                    if parity == 0:
                        t = work.tile([128, 2, SW], F8E5, tag="pt", bufs=4,
                                      name="pt")
                        ptp.append(t)
                    if fp8_pair:
                        # Schraudolph fast-exp on the DVE: affine map into
                        # the e5m2 bit pattern (uint8 convert saturates);
                        # splits the exp load across ACT and DVE
                        nc.vector.tensor_scalar(
                            ptp[p][:, parity, :].bitcast(U8), ps,
                            EXPA, EXPB, op0=Mult, op1=Add)
                    else:
                        nc.scalar.activation(ptp[p][:, parity, :], ps, Exp,
                                             scale=SCALE, bias=nshift)
                    if parity == 1 and p > 0:
                        drain_pair(p - 1)
                    if j8 == 2:
                        for fn in deferred:
                            fn()
                        deferred.clear()
                    if j8 == 8:
                        for fn in deferred2:
                            fn()
                        deferred2.clear()
                drain_pair(NPAIR - 1)

                # ---- finalize slice s: normalize + project + out ----
                def emit_otr(off, wdt, s=s, po=po):
                    otr = [work.tile([128, wdt], BF16, tag=f"otr{cc}",
                                     bufs=2, name=f"otr{cc}")
                           for cc in range(2)]
                    # po is PSUM: only DVE/ACT may read it; keep ACT free
                    nc.vector.tensor_copy(otr[0], po[0][:, off:off + wdt])
                    nc.vector.tensor_copy(otr[1], po[1][:, off:off + wdt])
                    return otr

                def make_finalize(off, wdt, otr, s=s, pb=pb, dma_engs=None):
                    state = {}

                    def fin_cc(cc):
                        sl2 = slice(s * SW + off, s * SW + off + wdt)
                        pf = pp.tile([128, wdt], F32, tag="sim", bufs=3,
                                     name="pf")
                        for ci in range(2):
                            mm(pf, w2t[ci][:, cc * 128:(cc + 1) * 128],
                               otr[ci], start=(ci == 0), stop=(ci == 1))
                        fo = work.tile([128, wdt], F32, tag="fo", bufs=2,
                                       name="fo")
                        nc.vector.tensor_mul(fo, pf, state["bc"])
                        fo2 = work.tile([128, wdt], BF16, tag="fo2", bufs=2,
                                        name="fo2")
                        nc.vector.scalar_tensor_tensor(
                            fo2, xr[cc][:, sl2], bt[cc], fo,
                            op0=Add, op1=Add)
                        deng = dma_engs[cc] if dma_engs else nc.sync
                        deng.dma_start(out[cc * 128:(cc + 1) * 128, sl2], fo2)

                    def fin_a():
                        bc = work.tile([128, wdt], F32, tag="bc", bufs=2,
                                       name="bc")
                        rsc = work.tile([128, wdt], F32, tag="rsc", bufs=2,
                                        name="rsc")
                        nc.vector.reciprocal_approx_accurate(
                            bc, pb[:, off:off + wdt], rsc)
                        state["bc"] = bc
                        fin_cc(0)

                    def fin_b():
                        fin_cc(1)
                    return fin_a, fin_b

                if not tail:
                    otr = emit_otr(0, SW)
                    fa, fb = make_finalize(0, SW, otr)
                    deferred.append(fa)
                    deferred2.append(fb)
                else:
                    # kernel tail: two 256-col halves so the first half's
                    # projection/output overlaps the second's
                    hwd = SW // 2
                    otr0 = emit_otr(0, hwd)
                    otr1 = emit_otr(hwd, hwd)
                    a0, b0 = make_finalize(0, hwd, otr0,
                                           dma_engs=(nc.scalar, nc.sync))
                    a1, b1 = make_finalize(hwd, hwd, otr1,
                                           dma_engs=(nc.gpsimd, nc.scalar))
                    a0(); b0(); a1(); b1()

    nc.finalize()
    return nc


_NC_CACHE = None


def _get_nc():
    global _NC_CACHE
    if _NC_CACHE is None:
        _NC_CACHE = build_nc()
    return _NC_CACHE


def prepare_in_maps(x, w_qkv, w_out, b_out):
    x = np.asarray(x, dtype=np.float32)
    w_qkv = np.asarray(w_qkv, dtype=np.float32)
    w_out = np.asarray(w_out, dtype=np.float32)
    b_out = np.asarray(b_out, dtype=np.float32)

    import ml_dtypes
    bf16 = ml_dtypes.bfloat16
    f8e4 = ml_dtypes.float8_e4m3
    wq, wk, wv = w_qkv[:512], w_qkv[512:1024], w_qkv[1024:]
    M = (wq.T.astype(np.float64) @ wk.astype(np.float64)).astype(np.float32)
    W2 = (w_out.astype(np.float64) @ wv.astype(np.float64)).T.astype(np.float32)
    m_bf = np.ascontiguousarray(M).astype(bf16)
    w2_bf = np.ascontiguousarray(W2).astype(bf16)
    bout = np.ascontiguousarray(b_out.reshape(2, 128, 1))

    xr = x.reshape(B, C, N)
    in_maps = []
    for c in range(NCORES):
        b, h = divmod(c, 2)
        if h == 0:
            x_rot = xr[b]
        else:  # rotate so this core's query half sits in columns 0:HALF
            x_rot = np.concatenate([xr[b][:, HALF:], xr[b][:, :HALF]], axis=1)
        x_bf = x_rot.astype(bf16)
        # token-pair planes: xp[p, pair, plane, c] = xf[(pair*2+plane)*128+p, c]
        xf8 = np.ascontiguousarray(x_bf.T).astype(f8e4)          # [N, C]
        xp = np.ascontiguousarray(
            xf8.reshape(NPAIR, 2, 128, C).transpose(2, 0, 1, 3))  # [128,16,2,C]
        # c-pair planes: xq8[p, r, j] = fp8(x_rot[r*128+p, j])
        xq8 = np.ascontiguousarray(
            x_bf.astype(f8e4).reshape(2, 128, N).transpose(1, 0, 2))
        in_maps.append({
            "x_r": x_bf,
            "xq8": xq8,
            "xp": xp,
            "m": m_bf,
            "w2": w2_bf,
            "bout": bout,
        })
    return in_maps


def postprocess(results):
    outs = [results[c]["out"] for c in range(NCORES)]
    full = np.stack([np.concatenate([outs[2 * b], outs[2 * b + 1]], axis=1)
                     for b in range(B)])               # [B, C, N]
    return full.reshape(B, C, 64, 64).astype(np.float32)


def kernel(x, w_qkv, w_out, b_out):
    in_maps = prepare_in_maps(x, w_qkv, w_out, b_out)
    res = run_bass_kernel_spmd(_get_nc(), in_maps, core_ids=list(range(NCORES)))
    return postprocess(res.results)
